# revision 1
# baseline (speedup 1.0000x reference)
"""Causal dilated conv1d (K=3, dilation=2, N=128 channels) on Trainium2.

out[b,t,i] = sum_{j,k} x[b, t-2k, j] * weight[i,j,k] + bias[i]

Strategy (8-core SPMD, pure data parallel over batch):
  - each core handles 4 of the 32 batch rows; weight/bias replicated
  - on-chip, per batch row: PE-transpose x into a [128(j), T+4] "strip"
    (4-col zero halo on the left so the dilated taps become plain column
    offsets), then 3 accumulated float32r matmuls with 512-wide moving
    operand compute out_T[i, t] = sum_k w_k^T @ xT[:, t-2k], ACT adds the
    per-partition bias while copying PSUM->SBUF, and PE transposes the
    result back to [t, i] layout for contiguous DMA out.
"""

import os
import threading

import numpy as np

import concourse.bass as bass  # noqa: F401  (bass types used via bacc/tile)
import concourse.mybir as mybir
import concourse.tile as tile
from concourse import bacc
from concourse.bass_utils import run_bass_kernel_spmd
from concourse.masks import make_identity

P = 128
KTAPS = 3
DIL = 2
HALO = (KTAPS - 1) * DIL  # 4
NCORES = 8
B_FULL, T_FULL = 32, 8192
B_CORE = B_FULL // NCORES  # 4

FP32 = mybir.dt.float32


def build(Bc=B_CORE, T=T_FULL, chunk=2048, tap_dtype=mybir.dt.float32r):
    """Build the per-core Bass module. Same NEFF runs SPMD on all 8 cores."""
    nc = bacc.Bacc(
        "TRN2",
        target_bir_lowering=False,
        debug=False,
        enable_asserts=False,
        num_devices=NCORES,
    )
    x_d = nc.dram_tensor("x", [Bc, T, P], tap_dtype, kind="ExternalInput")
    w_d = nc.dram_tensor("w", [P, KTAPS * P], tap_dtype, kind="ExternalInput")
    b_d = nc.dram_tensor("b", [P, 1], FP32, kind="ExternalInput")
    o_d = nc.dram_tensor("o", [Bc, T, P], FP32, kind="ExternalOutput")

    x_ap, o_ap = x_d.ap(), o_d.ap()
    n_chunks = T // chunk
    SW = 512  # tap-matmul moving width (1 PSUM bank of fp32)
    S = chunk // SW  # strips per chunk
    CPS = SW // P  # 128-subtiles per strip

    with tile.TileContext(nc) as tc:
        with (
            tc.tile_pool(name="const", bufs=1) as cp,
            tc.tile_pool(name="xn", bufs=3) as xp,
            tc.tile_pool(name="strip", bufs=2) as sp,
            tc.tile_pool(name="oT", bufs=3) as otp,
            tc.tile_pool(name="oc", bufs=3) as ocp,
            tc.tile_pool(name="pxt", bufs=3, space="PSUM") as pxtp,
            tc.tile_pool(name="pacc", bufs=3, space="PSUM") as paccp,
            tc.tile_pool(name="pto", bufs=2, space="PSUM") as ptop,
        ):
            ident = cp.tile([P, P], FP32)
            make_identity(nc, ident)
            # f32r copy of the identity for the (faster) f32r transpose-in path;
            # produced via DVE copy since memset/affine_select can't emit f32r.
            ident_r = cp.tile([P, P], tap_dtype)
            nc.vector.tensor_copy(ident_r[:], ident[:])
            w_sb = cp.tile([P, KTAPS * P], tap_dtype)
            nc.sync.dma_start(w_sb[:], w_d.ap())
            bias_sb = cp.tile([P, 1], FP32)
            nc.sync.dma_start(bias_sb[:], b_d.ap())
            zhalo = cp.tile([P, HALO], FP32)
            nc.vector.memset(zhalo[:], 0.0)

            R = chunk // P  # out rows per partition in the contiguous store

            # one-chunk-delayed transpose-out state: (oTv, b, t0) of the chunk
            # whose [t,i]-restore is interleaved into the NEXT chunk's strip
            # loop, so the PE never stalls waiting for the current chunk's
            # PSUM->SBUF bias copies (in-order engine streams).
            pending = None
            oc_pending = None

            def emit_tout_group(g):
                nonlocal oc_pending
                oTv_p, b_p, t0_p = pending
                if g == 0:
                    oc_pending = ocp.tile([P, chunk], FP32, tag="oc")
                pto = ptop.tile([P, SW], FP32, tag="pto")
                for c in range(CPS):
                    r = g * CPS + c
                    nc.tensor.transpose(
                        pto[:, c * P : (c + 1) * P], oTv_p[:, r, :], ident
                    )
                if g % 2 == 0:
                    nc.scalar.copy(oc_pending[:, g * SW : (g + 1) * SW], pto[:])
                else:
                    nc.vector.tensor_copy(
                        oc_pending[:, g * SW : (g + 1) * SW], pto[:]
                    )

            def emit_out_dma():
                _, b_p, t0_p = pending
                nc.sync.dma_start(
                    o_ap[b_p, t0_p : t0_p + chunk, :].rearrange(
                        "(p f) j -> p (f j)", p=P
                    ),
                    oc_pending[:],
                )

            for b in range(Bc):
                strip = sp.tile([P, T + HALO], tap_dtype, tag="strip")
                nc.vector.tensor_copy(strip[:, 0:HALO], zhalo[:])
                for ci in range(n_chunks):
                    t0 = ci * chunk
                    # load so partition p holds x rows {t0+c*128+p}: consecutive-t
                    # 128-blocks feed the transposes directly. Split the very
                    # first load per strip so the PE can start sooner.
                    xn = xp.tile([P, chunk], tap_dtype, tag="xn")
                    if b == 0 and ci == 0:
                        for s in range(S):
                            nc.sync.dma_start(
                                xn[:, s * SW : (s + 1) * SW].rearrange(
                                    "p (c j) -> p c j", j=P
                                ),
                                x_ap[b, t0 + s * SW : t0 + (s + 1) * SW, :].rearrange(
                                    "(c p) j -> p c j", p=P
                                ),
                            )
                    else:
                        nc.sync.dma_start(
                            xn.rearrange("p (c j) -> p c j", j=P),
                            x_ap[b, t0 : t0 + chunk, :].rearrange(
                                "(c p) j -> p c j", p=P
                            ),
                        )
                    # out_T accumulator for the whole chunk: [i, t-t0]
                    oT = otp.tile([P, chunk], FP32, tag="oT")
                    for s in range(S):
                        st = t0 + s * SW
                        # --- transpose x subtiles into the strip ---
                        pxt = pxtp.tile([P, SW], tap_dtype, tag="pxt")
                        for c in range(CPS):
                            cc = s * CPS + c
                            nc.tensor.transpose(
                                pxt[:, c * P : (c + 1) * P],
                                xn[:, cc * P : (cc + 1) * P],
                                ident_r,
                            )
                        nc.vector.tensor_copy(
                            strip[:, HALO + st : HALO + st + SW], pxt[:]
                        )
                        # --- 3 dilated taps, accumulated in PSUM ---
                        pacc = paccp.tile([P, SW], FP32, tag="pacc")
                        for k in range(KTAPS):
                            off = HALO + st - DIL * k
                            nc.tensor.matmul(
                                pacc[:],
                                w_sb[:, k * P : (k + 1) * P],
                                strip[:, off : off + SW],
                                start=(k == 0),
                                stop=(k == KTAPS - 1),
                            )
                        # --- bias during PSUM->SBUF copy (bias is per-partition here) ---
                        nc.scalar.add(oT[:, s * SW : (s + 1) * SW], pacc[:], bias_sb[:])
                        # --- delayed transpose-out of the PREVIOUS chunk ---
                        if pending is not None:
                            emit_tout_group(s)
                    if pending is not None:
                        emit_out_dma()
                    # transposed-out restore of this chunk happens during the
                    # next chunk's strip loop (col of oT = p*R + r)
                    pending = (oT.rearrange("n (p r) -> n r p", p=P), b, t0)
            # epilogue: restore + store the final chunk
            for g in range(S):
                emit_tout_group(g)
            emit_out_dma()
    nc.compile()
    return nc


_cache = {}
_lock = threading.Lock()


def _get_nc():
    with _lock:
        if "nc" not in _cache:
            tap = os.environ.get("CONV_TAP_DTYPE", "float32r")
            _cache["nc"] = build(tap_dtype=getattr(mybir.dt, tap))
        return _cache["nc"]


def prep_inputs(x, weight, bias):
    # w_all[j, k*128 + i] = weight[i, j, k]
    w_all = np.ascontiguousarray(
        np.transpose(np.asarray(weight, np.float32), (1, 2, 0)).reshape(P, KTAPS * P)
    )
    b2 = np.ascontiguousarray(np.asarray(bias, np.float32).reshape(P, 1))
    return np.ascontiguousarray(np.asarray(x, np.float32)), w_all, b2


def kernel(x, weight, bias, _trace=False):
    x, w_all, b2 = prep_inputs(x, weight, bias)
    nc = _get_nc()
    in_maps = [
        {"x": x[c * B_CORE : (c + 1) * B_CORE], "w": w_all, "b": b2}
        for c in range(NCORES)
    ]
    res = run_bass_kernel_spmd(nc, in_maps, core_ids=list(range(NCORES)), trace=_trace)
    out = np.concatenate([r["o"] for r in res.results], axis=0)
    if _trace:
        kernel.last_results = res
    return out



# revision 2
# speedup vs baseline: 1.0413x; 1.0413x over previous
"""Causal dilated conv1d (K=3, dilation=2, N=128 channels) on Trainium2.

out[b,t,i] = sum_{j,k} x[b, t-2k, j] * weight[i,j,k] + bias[i]

Strategy (8-core SPMD, pure data parallel over batch; bf16 datapath):
  - each core handles 4 of the 32 batch rows; weight/bias replicated.
  - x is converted to bf16 on host; per batch row the [T,128] slab is
    loaded DIRECTLY TRANSPOSED into SBUF as a [128(j), T] strip via the
    DMA xbar transpose engine (16x128 tiles, ~90% of DMA bandwidth, big
    contiguous DRAM reads) -- no PE transpose-in, no tiny descriptors.
  - taps: 3 accumulated bf16 matmuls per 512-wide PSUM window, moving
    operand = strip shifted by 2k columns. The causal left edge of each
    batch row is handled with narrowed matmuls at a psum column offset
    (t<0 contributes nothing), so the strip needs no zero halo.
  - ACT adds per-partition bias while copying PSUM->SBUF (bf16 out).
  - PE transposes the [i,t] result back to [t,i] using a stride-R moving
    operand so each output partition holds R consecutive t rows ->
    4 KB contiguous store descriptors. Output is bf16; host upconverts.
"""

import os
import threading

import numpy as np
import ml_dtypes

import concourse.bass as bass  # noqa: F401  (bass types used via bacc/tile)
import concourse.mybir as mybir
import concourse.tile as tile
from concourse import bacc
from concourse.bass_utils import run_bass_kernel_spmd
from concourse.masks import make_identity

P = 128
KTAPS = 3
DIL = 2
NCORES = 8
B_FULL, T_FULL = 32, 8192
B_CORE = B_FULL // NCORES  # 4

FP32 = mybir.dt.float32
BF16 = mybir.dt.bfloat16


def build(Bc=B_CORE, T=T_FULL, chunk=2048):
    """Build the per-core Bass module. Same NEFF runs SPMD on all 8 cores."""
    nc = bacc.Bacc(
        "TRN2",
        target_bir_lowering=False,
        debug=False,
        enable_asserts=False,
        num_devices=NCORES,
    )
    x_d = nc.dram_tensor("x", [Bc, T, P], BF16, kind="ExternalInput")
    w_d = nc.dram_tensor("w", [P, KTAPS * P], BF16, kind="ExternalInput")
    b_d = nc.dram_tensor("b", [P, 1], FP32, kind="ExternalInput")
    o_d = nc.dram_tensor("o", [Bc, T, P], BF16, kind="ExternalOutput")

    x_ap, o_ap = x_d.ap(), o_d.ap()
    n_chunks = T // chunk
    SW = 512  # tap-matmul moving width (1 PSUM bank of fp32)
    S = chunk // SW  # strips per chunk
    R = chunk // P  # consecutive t rows per output partition

    with tile.TileContext(nc) as tc:
        with (
            tc.tile_pool(name="const", bufs=1) as cp,
            tc.tile_pool(name="strip", bufs=2) as sp,
            tc.tile_pool(name="oT", bufs=3) as otp,
            tc.tile_pool(name="oc", bufs=3) as ocp,
            tc.tile_pool(name="pacc", bufs=3, space="PSUM") as paccp,
            tc.tile_pool(name="pto", bufs=3, space="PSUM") as ptop,
        ):
            ident = cp.tile([P, P], FP32)
            make_identity(nc, ident)
            ident_bf = cp.tile([P, P], BF16)
            nc.vector.tensor_copy(ident_bf[:], ident[:])
            w_sb = cp.tile([P, KTAPS * P], BF16)
            nc.sync.dma_start(w_sb[:], w_d.ap())
            bias_sb = cp.tile([P, 1], FP32)
            nc.sync.dma_start(bias_sb[:], b_d.ap())

            # one-chunk-delayed transpose-out state: (oT, b, c) of the chunk
            # whose [t,i]-restore is interleaved behind the NEXT chunk's taps
            # so the PE never stalls waiting on ACT's PSUM->SBUF bias copies.
            pending = None
            oc_pending = None

            def emit_tout_group(g):
                nonlocal oc_pending
                oT_p, b_p, c_p = pending
                if g == 0:
                    oc_pending = ocp.tile([P, chunk], BF16, tag="oc")
                # column q + R*p of oT_p holds t = c*chunk + p*R + q
                oTv = oT_p.rearrange("n (p q) -> n q p", p=P)
                pto = ptop.tile([P, SW], BF16, tag="pto")
                for qq in range(4):
                    q = g * 4 + qq
                    nc.tensor.transpose(
                        pto[:, qq * P : (qq + 1) * P], oTv[:, q, :], ident_bf
                    )
                if g % 2 == 0:
                    nc.scalar.copy(oc_pending[:, g * SW : (g + 1) * SW], pto[:])
                else:
                    nc.vector.tensor_copy(
                        oc_pending[:, g * SW : (g + 1) * SW], pto[:]
                    )

            def emit_out_dma():
                _, b_p, c_p = pending
                nc.sync.dma_start(
                    o_ap[b_p, c_p * chunk : (c_p + 1) * chunk, :].rearrange(
                        "(p r) i -> p (r i)", p=P
                    ),
                    oc_pending[:],
                )

            for b in range(Bc):
                strip = sp.tile([P, T], BF16, tag="strip")
                for ci in range(n_chunks):
                    t0 = ci * chunk
                    # xbar-transposed load: [chunk, 128] DRAM -> [128, chunk]
                    nc.sync.dma_start_transpose(
                        strip[:, t0 : t0 + chunk],
                        x_ap[b, t0 : t0 + chunk, :],
                    )
                    for s in range(S):
                        st = t0 + s * SW
                        pacc = paccp.tile([P, SW], FP32, tag="pacc")
                        if st == 0:
                            # causal left edge: t-2k < 0 contributes nothing
                            nc.tensor.matmul(
                                pacc[:], w_sb[:, 0:P], strip[:, 0:SW],
                                start=True, stop=False,
                            )
                            nc.tensor.matmul(
                                pacc[:, 2:SW], w_sb[:, P : 2 * P],
                                strip[:, 0 : SW - 2],
                                start=False, stop=False,
                            )
                            nc.tensor.matmul(
                                pacc[:, 4:SW], w_sb[:, 2 * P : 3 * P],
                                strip[:, 0 : SW - 4],
                                start=False, stop=True,
                            )
                        else:
                            for k in range(KTAPS):
                                off = st - DIL * k
                                nc.tensor.matmul(
                                    pacc[:],
                                    w_sb[:, k * P : (k + 1) * P],
                                    strip[:, off : off + SW],
                                    start=(k == 0),
                                    stop=(k == KTAPS - 1),
                                )
                        if s == 0:
                            oT = otp.tile([P, chunk], BF16, tag="oT")
                        # bias add during PSUM->SBUF copy (bias per-partition)
                        nc.scalar.add(oT[:, s * SW : (s + 1) * SW], pacc[:], bias_sb[:])
                        # delayed transpose-out of the PREVIOUS chunk
                        if pending is not None:
                            emit_tout_group(s)
                    if pending is not None:
                        emit_out_dma()
                    pending = (oT, b, ci)
            # epilogue: restore + store the final chunk
            for g in range(S):
                emit_tout_group(g)
            emit_out_dma()
    nc.compile()
    return nc


_cache = {}
_lock = threading.Lock()


def _get_nc():
    with _lock:
        if "nc" not in _cache:
            _cache["nc"] = build()
        return _cache["nc"]


def prep_inputs(x, weight, bias):
    # w_all[j, k*128 + i] = weight[i, j, k]
    w_all = np.ascontiguousarray(
        np.transpose(np.asarray(weight, np.float32), (1, 2, 0))
        .reshape(P, KTAPS * P)
        .astype(ml_dtypes.bfloat16)
    )
    b2 = np.ascontiguousarray(np.asarray(bias, np.float32).reshape(P, 1))
    xb = np.ascontiguousarray(np.asarray(x, np.float32).astype(ml_dtypes.bfloat16))
    return xb, w_all, b2


def kernel(x, weight, bias, _trace=False):
    x, w_all, b2 = prep_inputs(x, weight, bias)
    nc = _get_nc()
    in_maps = [
        {"x": x[c * B_CORE : (c + 1) * B_CORE], "w": w_all, "b": b2}
        for c in range(NCORES)
    ]
    res = run_bass_kernel_spmd(nc, in_maps, core_ids=list(range(NCORES)), trace=_trace)
    out = np.concatenate(
        [np.asarray(r["o"]).astype(np.float32) for r in res.results], axis=0
    )
    if _trace:
        kernel.last_results = res
    return out
